# revision 1
# baseline (speedup 1.0000x reference)
"""AttentionDecoderModel (2-layer GCLSTM + additive attention) on 8 NeuronCores.

Data-parallel over batch (64 -> 8 per core), supports/weights replicated,
per the sharding hint. Executed via jax/PJRT on the 8 axon NeuronCores with
GSPMD sharding constraints; all compute is local to each core (no collectives).
"""
import numpy as np
import jax
import jax.numpy as jnp
from jax.sharding import Mesh, PartitionSpec as P, NamedSharding

K = 2          # max_diffusion_step
N_CORES = 8

_state = {}


def _diffusion_conv(x, support, W, b):
    x0 = x
    x1 = jnp.einsum('nm,bmd->bnd', support, x0)
    xs = [x0, x1]
    for _ in range(K - 1):
        xs.append(2.0 * jnp.einsum('nm,bmd->bnd', support, xs[-1]) - xs[-2])
    feats = jnp.concatenate(xs, axis=-1)
    return feats @ W + b


def _gclstm_cell(x, h, c, support, W, b):
    gates = _diffusion_conv(jnp.concatenate([x, h], axis=-1), support, W, b)
    i, f, o, g = jnp.split(gates, 4, axis=-1)
    c_new = jax.nn.sigmoid(f) * c + jax.nn.sigmoid(i) * jnp.tanh(g)
    h_new = jax.nn.sigmoid(o) * jnp.tanh(c_new)
    return h_new, c_new


def _model(inputs, encoder_outputs, hidden_state, cell_state, supports,
           w0, b0, w1, b1, attn_w, attn_b, proj_w, proj_b):
    out = inputs
    hs, cs = [], []
    for layer, (W, b) in enumerate([(w0, b0), (w1, b1)]):
        h_new, c_new = _gclstm_cell(out, hidden_state[layer], cell_state[layer],
                                    supports, W, b)
        hs.append(h_new)
        cs.append(c_new)
        out = h_new

    energy = jnp.tanh(encoder_outputs @ attn_w + attn_b)           # (S,B,N,H)
    energy = jnp.sum(energy * out[None], axis=-1)                  # (S,B,N)
    attention_weights = jax.nn.softmax(jnp.transpose(energy, (1, 2, 0)), axis=-1)

    context = jnp.einsum('bns,sbnh->bnh', attention_weights, encoder_outputs)
    cat = jnp.concatenate([out, context], axis=-1)
    projected = cat @ proj_w + proj_b
    B = out.shape[0]
    output = projected.reshape(B, -1)
    return output, jnp.stack(hs), jnp.stack(cs), attention_weights


def _build():
    if "fn" in _state:
        return
    devices = jax.devices()[:N_CORES]
    mesh = Mesh(np.asarray(devices), ("b",))

    def spec(*ps):
        return NamedSharding(mesh, P(*ps))

    in_shardings = (
        spec("b"),                 # inputs (B,N,1)
        spec(None, "b"),           # encoder_outputs (S,B,N,H)
        spec(None, "b"),           # hidden_state (2,B,N,H)
        spec(None, "b"),           # cell_state (2,B,N,H)
        spec(),                    # supports (N,N) replicated
        spec(), spec(), spec(), spec(),   # w0 b0 w1 b1
        spec(), spec(), spec(), spec(),   # attn_w attn_b proj_w proj_b
    )
    out_shardings = (spec("b"), spec(None, "b"), spec(None, "b"), spec("b"))
    _state["mesh"] = mesh
    _state["in_shardings"] = in_shardings
    _state["fn"] = jax.jit(_model, in_shardings=in_shardings,
                           out_shardings=out_shardings)


_ORDER = ("inputs", "encoder_outputs", "hidden_state", "cell_state", "supports",
          "w0", "b0", "w1", "b1", "attn_w", "attn_b", "proj_w", "proj_b")


def prep_args(inputs):
    """Device-put full inputs with their shardings (host->device once)."""
    _build()
    return [jax.device_put(np.asarray(inputs[k]), s)
            for k, s in zip(_ORDER, _state["in_shardings"])]


def run_on_device(args):
    outs = _state["fn"](*args)
    jax.block_until_ready(outs)
    return outs


def kernel(**inputs):
    args = prep_args(inputs)
    outs = run_on_device(args)
    return tuple(np.asarray(o) for o in outs)


if __name__ == "__main__":
    rng = np.random.default_rng(0)
    demo = {
        "inputs": rng.standard_normal((64, 1024, 1)).astype(np.float32),
        "encoder_outputs": rng.standard_normal((12, 64, 1024, 128)).astype(np.float32),
        "hidden_state": rng.standard_normal((2, 64, 1024, 128)).astype(np.float32),
        "cell_state": rng.standard_normal((2, 64, 1024, 128)).astype(np.float32),
        "supports": rng.standard_normal((1024, 1024)).astype(np.float32),
        "w0": rng.standard_normal((387, 512)).astype(np.float32),
        "b0": np.zeros(512, np.float32),
        "w1": rng.standard_normal((768, 512)).astype(np.float32),
        "b1": np.zeros(512, np.float32),
        "attn_w": rng.standard_normal((128, 128)).astype(np.float32),
        "attn_b": np.zeros(128, np.float32),
        "proj_w": rng.standard_normal((256, 1)).astype(np.float32),
        "proj_b": np.zeros(1, np.float32),
    }
    outs = kernel(**demo)
    for o in outs:
        print(o.shape, o.dtype)
